# revision 1
# baseline (speedup 1.0000x reference)
"""V6 (deep-buffered, partition-major DMA layouts + grouped ACT sets): all-bf16 inputs, batched 3D-window DMAs, wide fused ops.

Differences vs V1:
- tpad and x are passed as bf16 (t is exactly representable; x rounding adds
  ~1e-6 statistical noise to the mean).
- The row-aligned t needed for the sign-flip is a second DMA *view* of tpad
  (offset by the 2-row/2-col pad), so no shift matmul and no PSUM pressure.
- DMAs are batched: one 3D-AP transfer covers all 4 main windows of an image
  (and one covers the 4 tail windows of all 4 images).
- Elementwise/ACT ops run 2-window wide (1024 free-dim) or 4-window wide
  (softplus: 2048) to amortize fixed costs; the whole y path is uint16:
  y = (t_u16 << 8) XOR x_u16  (bf16 1.0 == 0x3F80 -> 0x8000 sign bit).
"""

import numpy as np

import concourse.bass as bass
import concourse.bacc as bacc_mod
import concourse.tile as tile
from concourse import mybir
from concourse.bass_utils import run_bass_kernel_spmd
from concourse.tile import add_dep_helper

F32 = mybir.dt.float32
BF16 = mybir.dt.bfloat16
U16 = mybir.dt.uint16
ALU = mybir.AluOpType
ACTF = mybir.ActivationFunctionType

B, H, W = 32, 512, 512
NCORES = 8
IMGS = B // NCORES
PAD = 2
TP = H + 2 * PAD            # 516
NWIN = 5
# main windows (4): t rows [124w, 124w+128), out rows [124w, 124w+124) at
# partitions [0,124).  tail: t rows [388, 516), out rows [496, 512) at
# partitions [0,16).
NMAIN = 4
TAIL_IS = 388
TAIL_NPW = 16
TAIL_G0 = 496


def _make_bands() -> np.ndarray:
    bands = np.zeros((2, 128, 124), dtype=np.float32)
    for m in range(124):
        bands[0, m: m + 5, m] = 1.0
    for m in range(16):
        bands[1, 108 + m: 113 + m, m] = 1.0
    return bands


def _ap3(t, off, dims):
    return bass.AP(t, off, dims)


def _build_nc() -> bass.Bass:
    nc = bacc_mod.Bacc(trn_type="TRN2")

    # host pre-arranged, partition-major inputs (contiguous per partition)
    twin = nc.dram_tensor("twin", [IMGS, 128, NWIN, TP], BF16, kind="ExternalInput")
    xw = nc.dram_tensor("xw", [IMGS, 124, NMAIN, W], BF16, kind="ExternalInput")
    taw = nc.dram_tensor("taw", [IMGS, 124, NMAIN, W], BF16, kind="ExternalInput")
    xtail = nc.dram_tensor("xtail", [IMGS, TAIL_NPW, W], BF16, kind="ExternalInput")
    tatail = nc.dram_tensor("tatail", [IMGS, TAIL_NPW, W], BF16, kind="ExternalInput")
    band = nc.dram_tensor("band", [2, 128, 124], BF16, kind="ExternalInput")
    out_sp = nc.dram_tensor("out_sp", [128, IMGS + 1], F32, kind="ExternalOutput")
    out_r = nc.dram_tensor("out_r", [128, 2 * IMGS + 2], F32, kind="ExternalOutput")

    IMG = TP * TP           # elements per padded image
    XIMG = H * W

    with tile.TileContext(nc) as tc:
        with (
            tc.tile_pool(name="singles", bufs=1) as singles,
            tc.tile_pool(name="tin", bufs=2) as tin,
            tc.tile_pool(name="xin", bufs=3) as xin,
            tc.tile_pool(name="tain", bufs=3) as tain,
            tc.tile_pool(name="vp", bufs=2) as vp,
            tc.tile_pool(name="yp", bufs=3) as yp,
            tc.tile_pool(name="spp", bufs=3) as spp,
            tc.tile_pool(name="eyp", bufs=2) as eyp,
            tc.tile_pool(name="uap", bufs=3) as uap,
            tc.tile_pool(name="scrp", bufs=3) as scrp,
            tc.tile_pool(name="mkp", bufs=3) as mkp,
            tc.tile_pool(name="psum", bufs=4, space="PSUM") as psum,
        ):
            band_sb = singles.tile([128, 2 * 124], BF16)
            nc.sync.dma_start(band_sb[:, 0:124], band[0])
            nc.sync.dma_start(band_sb[:, 124:248], band[1])
            bias_abs = singles.tile([128, 1], F32)
            nc.gpsimd.memset(bias_abs[:], -12.5)

            # integer shift amount for the bitvec stt (imm floats are rejected)
            shift8 = singles.tile([128, 1], U16)
            nc.vector.memset(shift8[:], 8)

            stats_sp = singles.tile([128, IMGS + 1], F32)
            stats_r = singles.tile([128, 2 * IMGS + 2], F32)
            nc.vector.memset(stats_sp[:], 0.0)
            nc.vector.memset(stats_r[:], 0.0)

            v_tl_t = singles.tile([128, IMGS, TP - 2], BF16)

            # ---- tail input tiles, batched across the 4 images ----
            t_tl = singles.tile([128, IMGS, TP], BF16)
            nc.sync.dma_start(
                t_tl[:],
                _ap3(twin, 4 * TP,
                     [[NWIN * TP, 128], [128 * NWIN * TP, IMGS], [1, TP]]),
            )
            x_tl = singles.tile([TAIL_NPW, IMGS, W], BF16)
            nc.sync.dma_start(
                x_tl[:],
                _ap3(xtail, 0, [[W, TAIL_NPW], [TAIL_NPW * W, IMGS], [1, W]]),
            )
            ta_tl = singles.tile([TAIL_NPW, IMGS, W], BF16)
            nc.sync.dma_start(
                ta_tl[:],
                _ap3(tatail, 0, [[W, TAIL_NPW], [TAIL_NPW * W, IMGS], [1, W]]),
            )

            # persistent cross-phase tiles (phase 2 reads them)
            t_all = singles.tile([128, IMGS, NMAIN, TP], BF16)
            v_all = singles.tile([128, IMGS, NMAIN, TP - 2], BF16)
            ey_all = singles.tile([124, IMGS, NMAIN, W], F32)
            ey_tl = singles.tile([TAIL_NPW, IMGS, W], F32)

            exp_insts = []

            # ---- phase 1: loads, box pair-sums, sign-flip, EXP (one ACT set)
            for im in range(IMGS):
                t_w = t_all[:, im, :, :]
                nc.sync.dma_start(
                    t_w,
                    _ap3(twin, im * 128 * NWIN * TP,
                         [[NWIN * TP, 128], [TP, NMAIN], [1, TP]]),
                )
                x_w = xin.tile([124, NMAIN, W], BF16, tag="x")
                nc.sync.dma_start(
                    x_w[:],
                    _ap3(xw, im * 124 * NMAIN * W,
                         [[NMAIN * W, 124], [W, NMAIN], [1, W]]),
                )
                ta_w = tain.tile([124, NMAIN, W], BF16, tag="ta")
                nc.sync.dma_start(
                    ta_w[:],
                    _ap3(taw, im * 124 * NMAIN * W,
                         [[NMAIN * W, 124], [W, NMAIN], [1, W]]),
                )

                nc.vector.tensor_tensor(
                    v_all[:, im, :, :], t_w[:, :, 0: TP - 2], t_w[:, :, 2:TP],
                    op=ALU.add,
                )

                mk = mkp.tile([124, NMAIN, W], BF16, tag="mk")
                nc.vector.tensor_scalar(
                    mk[:].bitcast(U16),
                    ta_w[:].bitcast(U16),
                    8,
                    None,
                    op0=ALU.logical_shift_left,
                )
                y_w = yp.tile([124, NMAIN, W], BF16, tag="y")
                nc.vector.tensor_tensor(
                    y_w[:].bitcast(U16),
                    mk[:].bitcast(U16),
                    x_w[:].bitcast(U16),
                    op=ALU.bitwise_xor,
                )
                exp_insts.append(nc.scalar.activation(ey_all[:, im, :, :], y_w[:], ACTF.Exp))

            # phase-1 tail: sign-flip + EXP while the exp set is loaded
            nc.vector.tensor_tensor(
                v_tl_t[:], t_tl[:, :, 0: TP - 2], t_tl[:, :, 2:TP], op=ALU.add
            )
            mk_tl = mkp.tile([TAIL_NPW, IMGS, W], BF16, tag="mk")
            nc.vector.tensor_scalar(
                mk_tl[:].bitcast(U16),
                ta_tl[:].bitcast(U16),
                8,
                None,
                op0=ALU.logical_shift_left,
            )
            y_tl = yp.tile([TAIL_NPW, IMGS, W], BF16, tag="y")
            nc.vector.tensor_tensor(
                y_tl[:].bitcast(U16),
                mk_tl[:].bitcast(U16),
                x_tl[:].bitcast(U16),
                op=ALU.bitwise_xor,
            )
            exp_insts.append(nc.scalar.activation(ey_tl[:], y_tl[:], ACTF.Exp))

            # ---- phase 2: LN (+accum), box matmuls, ABS, weighted term
            # (ln and abs share the natural_log set -> no more table loads)
            for im in range(IMGS):
                spy_w = spp.tile([124, NMAIN, W], BF16, tag="spy")
                ln_i = nc.scalar.activation(
                    spy_w[:],
                    ey_all[:, im, :, :],
                    ACTF.Ln,
                    bias=1.0,
                    accum_out=stats_sp[0:124, im: im + 1],
                )
                for g in range(NMAIN // 2):
                    s_ps = psum.tile([128, 2, W], F32, tag="s")
                    for j in range(2):
                        wgt = band_sb[:, 0:124]
                        wv = v_all[:, im, 2 * g + j, :]
                        wt = t_all[:, im, 2 * g + j, :]
                        nc.tensor.matmul(
                            s_ps[0:124, j, :], wgt, wv[:, 0:W],
                            start=True, stop=False,
                        )
                        nc.tensor.matmul(
                            s_ps[0:124, j, :], wgt, wv[:, 1: W + 1],
                            start=False, stop=False,
                        )
                        nc.tensor.matmul(
                            s_ps[0:124, j, :], wgt, wt[:, 4: W + 4],
                            start=False, stop=True,
                        )

                    uab = uap.tile([124, 2, W], BF16, tag="uab")
                    nc.scalar.activation(
                        uab[:],
                        s_ps[0:124, :, :],
                        ACTF.Abs,
                        bias=bias_abs[0:124, :],
                    )
                    scr = scrp.tile([124, 2, W], BF16, tag="scr")
                    nc.vector.scalar_tensor_tensor(
                        scr[:],
                        uab[:],
                        12.0,
                        spy_w[:, 2 * g: 2 * g + 2, :],
                        op0=ALU.is_gt,
                        op1=ALU.mult,
                        accum_out=stats_r[0:124, 2 * im + g: 2 * im + g + 1],
                    )

            # ---- tail phase 2 ----
            spy_tl = spp.tile([TAIL_NPW, IMGS, W], BF16, tag="spy")
            ln_i = nc.scalar.activation(
                spy_tl[:],
                ey_tl[:],
                ACTF.Ln,
                bias=1.0,
                accum_out=stats_sp[0:TAIL_NPW, IMGS: IMGS + 1],
            )
            for g in range(IMGS // 2):
                s_ps = psum.tile([128, 2, W], F32, tag="s")
                for j in range(2):
                    im = 2 * g + j
                    wgt = band_sb[:, 124: 124 + 16]
                    nc.tensor.matmul(
                        s_ps[0:TAIL_NPW, j, :], wgt, v_tl_t[:, im, 0:W],
                        start=True, stop=False,
                    )
                    nc.tensor.matmul(
                        s_ps[0:TAIL_NPW, j, :], wgt, v_tl_t[:, im, 1: W + 1],
                        start=False, stop=False,
                    )
                    nc.tensor.matmul(
                        s_ps[0:TAIL_NPW, j, :], wgt, t_tl[:, im, 4: W + 4],
                        start=False, stop=True,
                    )
                uab = uap.tile([TAIL_NPW, 2, W], BF16, tag="uab")
                nc.scalar.activation(
                    uab[:],
                    s_ps[0:TAIL_NPW, :, :],
                    ACTF.Abs,
                    bias=bias_abs[0:TAIL_NPW, :],
                )
                scr = scrp.tile([TAIL_NPW, 2, W], BF16, tag="scr")
                nc.vector.scalar_tensor_tensor(
                    scr[:],
                    uab[:],
                    12.0,
                    spy_tl[:, 2 * g: 2 * g + 2, :],
                    op0=ALU.is_gt,
                    op1=ALU.mult,
                    accum_out=stats_r[0:TAIL_NPW, 2 * IMGS + g: 2 * IMGS + g + 1],
                )

            nc.sync.dma_start(out_sp[:], stats_sp[:])
            nc.sync.dma_start(out_r[:], stats_r[:])

    nc.compile()
    nc.finalize()
    return nc


_NC = None


def _get_nc() -> bass.Bass:
    global _NC
    if _NC is None:
        _NC = _build_nc()
    return _NC


def _make_in_maps(pred: np.ndarray, target: np.ndarray) -> list[dict]:
    import ml_dtypes

    bf16 = ml_dtypes.bfloat16
    pred = np.ascontiguousarray(pred.reshape(B, H, W)).astype(bf16)
    target = target.reshape(B, H, W)
    tpad = np.zeros((B, TP, TP), dtype=bf16)
    tpad[:, PAD: PAD + H, PAD: PAD + W] = target.astype(bf16)
    bands = _make_bands().astype(bf16)

    # partition-major window stacks so every DMA is contiguous per partition
    WIN_IS = [0, 124, 248, 372, TAIL_IS]
    twin = np.empty((B, 128, NWIN, TP), dtype=bf16)
    for w, is_ in enumerate(WIN_IS):
        twin[:, :, w, :] = tpad[:, is_: is_ + 128, :]
    main = lambda a: np.ascontiguousarray(
        a[:, 0: 4 * 124, :].reshape(B, NMAIN, 124, a.shape[2]).transpose(0, 2, 1, 3)
    )
    xw = main(pred)
    taw = main(target.astype(bf16))
    xtail = np.ascontiguousarray(pred[:, TAIL_G0:, :])
    tatail = np.ascontiguousarray(target[:, TAIL_G0:, :].astype(bf16))

    in_maps = []
    for c in range(NCORES):
        sl = slice(c * IMGS, (c + 1) * IMGS)
        in_maps.append(
            {
                "twin": np.ascontiguousarray(twin[sl]),
                "xw": np.ascontiguousarray(xw[sl]),
                "taw": np.ascontiguousarray(taw[sl]),
                "xtail": xtail[sl],
                "tatail": tatail[sl],
                "band": bands,
            }
        )
    return in_maps


def _finish(results: list[dict]) -> np.ndarray:
    total = 0.0
    for res in results:
        total += 5.0 * np.sum(res["out_sp"], dtype=np.float64)
        total -= 4.0 * np.sum(res["out_r"], dtype=np.float64)
    mean = total / float(B * H * W)
    return np.asarray(np.float32(mean))


def kernel(pred: np.ndarray, target: np.ndarray, **run_kwargs) -> np.ndarray:
    pred = np.asarray(pred)
    target = np.asarray(target)
    nc = _get_nc()
    in_maps = _make_in_maps(pred, target)
    out = run_bass_kernel_spmd(nc, in_maps, core_ids=list(range(NCORES)), **run_kwargs)
    res = _finish(out.results)
    kernel.last_run = out
    return res

